# revision 6
# baseline (speedup 1.0000x reference)
"""Multi-head attention forward (B=2, S=2048, D=1024, H=16, Hd=64) on 8
Trainium2 NeuronCores.

Sharding: core c handles batch b = c // 4 and 4 heads (c % 4) * 4 .. +4.
Each core computes its heads' Q/K/V projections, attention, and a partial
row-parallel o_proj; the host sums the 4 partial outputs per batch and adds
the output bias.

On-chip layout (per core, S=2048, Dloc=256 = 4 heads):
  - x.T tiles are materialized block-wise via PE transposes.
  - qhT/khT are kept head-pair-stacked: [128 (2 heads x 64), S] so all
    matmuls run with full 128-partition contraction where possible.
  - scores are computed transposed (scoresT[sk, q]) so softmax's sum over
    keys is the matmul contraction dim; the sum-of-exp comes for free from
    a ones-column appended to V (vh_aug, M=65).
  - softmax skips the max subtraction: scores ~ N(0,1) for this problem's
    scale, exp stays far from fp32 overflow.
  - matmul operands use float32r (full-rate fp32, ~1.5e-4 matmul rel-err).
"""

import numpy as np

S = 2048
D = 1024
H = 16
HD = 64
B = 2

P = 128
SBLK = 512          # s-positions per block
NSB = S // SBLK     # 4
DCH = D // P        # 8
NKT = S // P        # 16 key tiles
NH = 4              # heads per core
NG = 2              # head-pair groups per core
DLOC = NH * HD      # 256

_program_cache = {}


def _split_excess_waits(nc, mybir, max_waits=1):
    """This walrus build rejects instructions with >1 semaphore wait. Move
    excess waits onto preceding NoOps on the same engine queue (engines are
    strict FIFO, so blocking a NoOp blocks the instruction)."""
    n = 0
    for f in nc.m.functions:
        for bb in f.blocks:
            new = []
            changed = False
            for inst in bb.instructions:
                si = inst.sync_info
                waits = list(si.on_wait) if si is not None else []
                if len(waits) > max_waits:
                    extra = waits[:-max_waits]
                    keep = waits[-max_waits:]
                    for i in range(0, len(extra), max_waits):
                        nop = mybir.InstNoOp(
                            name=f"__waitsplit_{n}", ins=[], outs=[]
                        )
                        n += 1
                        nop.engine = inst.engine
                        nop.sync_info = mybir.SyncInfo(
                            on_wait=extra[i : i + max_waits], on_update=[]
                        )
                        new.append(nop)
                    inst.sync_info = mybir.SyncInfo(
                        on_wait=keep, on_update=list(si.on_update)
                    )
                    changed = True
                new.append(inst)
            if changed:
                bb.instructions = new
    return n


def _build_program():
    import concourse.bass as bass
    import concourse.mybir as mybir
    from concourse.bass import ds, ts
    from concourse.masks import make_identity
    from concourse.tile import TileContext

    f32 = mybir.dt.float32
    f32r = mybir.dt.float32r
    AF = mybir.ActivationFunctionType

    nc = bass.Bass()
    xq = nc.declare_dram_parameter("xq", [S, D], f32, isOutput=False)
    xk = nc.declare_dram_parameter("xk", [S, D], f32, isOutput=False)
    xv = nc.declare_dram_parameter("xv", [S, D], f32, isOutput=False)
    wq = nc.declare_dram_parameter("wq", [D, DLOC], f32, isOutput=False)
    wk = nc.declare_dram_parameter("wk", [D, DLOC], f32, isOutput=False)
    wv = nc.declare_dram_parameter("wv", [D, DLOC], f32, isOutput=False)
    wo = nc.declare_dram_parameter("wo", [DLOC, D], f32, isOutput=False)
    bq = nc.declare_dram_parameter("bq", [DLOC], f32, isOutput=False)
    bk = nc.declare_dram_parameter("bk", [DLOC], f32, isOutput=False)
    bv = nc.declare_dram_parameter("bv", [DLOC], f32, isOutput=False)
    y = nc.declare_dram_parameter("y", [S, D], f32, isOutput=True)

    with TileContext(nc) as tc:
        with (
            tc.tile_pool(name="const", bufs=1) as const,
            tc.tile_pool(name="wscratch", bufs=1) as wscratch,
            tc.tile_pool(name="kv", bufs=1) as kv,
            tc.tile_pool(name="xnat", bufs=2) as xnat,
            tc.tile_pool(name="xtp", bufs=2) as xtp,
            tc.tile_pool(name="qpool", bufs=2) as qpool,
            tc.tile_pool(name="epool", bufs=3) as epool,
            tc.tile_pool(name="cpool", bufs=2) as cpool,
            tc.tile_pool(name="rpool", bufs=2) as rpool,
            tc.tile_pool(name="opool", bufs=2) as opool,
            tc.tile_pool(name="ps_t", bufs=2, space="PSUM") as ps_t,
            tc.tile_pool(name="ps_k", bufs=1, space="PSUM") as ps_k,
            tc.tile_pool(name="ps_s", bufs=2, space="PSUM") as ps_s,
            tc.tile_pool(name="ps_c", bufs=1, space="PSUM") as ps_c,
        ):
            # ---- constants / weights -------------------------------------
            ident = const.tile([P, P], f32)
            make_identity(nc, ident)

            ones_col = const.tile([1, 64], f32)
            nc.gpsimd.memset(ones_col, 1.0)
            one_sb = const.tile([P, 1], f32)
            nc.gpsimd.memset(one_sb, 1.0)

            def load_weight_T(dram, name):
                nat = wscratch.tile([P, DCH, DLOC], f32, tag="wnat")
                nc.sync.dma_start(
                    nat[:], dram.rearrange("(dc p) n -> p dc n", p=P)
                )
                r = const.tile([P, DCH, DLOC], f32r, tag=f"w_{name}")
                nc.vector.tensor_copy(r[:], nat[:])
                return r

            wq_r = load_weight_T(wq, "q")
            wk_r = load_weight_T(wk, "k")
            wv_r = load_weight_T(wv, "v")

            wo_nat = wscratch.tile([P, NG, D], f32, tag="wnat")
            nc.sync.dma_start(
                wo_nat[:], wo.rearrange("(g p) n -> p g n", p=P)
            )
            wo_r = const.tile([P, NG, D], f32r)
            nc.vector.tensor_copy(wo_r[:], wo_nat[:])

            bq_sb = const.tile([P, NG], f32)
            nc.sync.dma_start(bq_sb[:], bq.rearrange("(g p) -> p g", p=P))
            bk_sb = const.tile([P, NG], f32)
            nc.sync.dma_start(bk_sb[:], bk.rearrange("(g p) -> p g", p=P))
            bv_sb = const.tile([P, NG], f32)
            nc.sync.dma_start(bv_sb[:], bv.rearrange("(g p) -> p g", p=P))

            # persistent K/V state
            khT = [
                kv.tile([P, S], f32r, tag=f"khT{g}", name=f"khT{g}")
                for g in range(NG)
            ]
            # vh_aug: [sk-part, kt, head*65] with col 64 of each head == 1.0
            vh_aug = kv.tile([P, NKT, NH * 65], f32r)
            vh4 = vh_aug[:].rearrange("p k (h e) -> p k h e", e=65)
            nc.vector.tensor_copy(
                vh4[:, :, :, 64], one_sb[:].to_broadcast([P, NKT, NH])
            )

            def transpose_block(src_dram, sb):
                """DMA a [512, D] row-block and produce its transpose
                [P, DCH, 512] (d on partitions) in f32r."""
                nat = xnat.tile([P, 4, D], f32, tag="xnat")
                nc.sync.dma_start(
                    nat[:],
                    src_dram[ds(sb * SBLK, SBLK), :].rearrange(
                        "(ss p) d -> p ss d", p=P
                    ),
                )
                xT = xtp.tile([P, DCH, SBLK], f32r, tag="xtp")
                for dc in range(DCH):
                    pt = ps_t.tile([P, SBLK], f32, tag="t")
                    for ss in range(4):
                        nc.tensor.transpose(
                            pt[:, ts(ss, P)],
                            nat[:, ss, ts(dc, P)],
                            ident,
                        )
                    nc.vector.tensor_copy(xT[:, dc, :], pt[:])
                return xT

            # ---- phase A: K/V projections --------------------------------
            for sb in range(NSB):
                xkT = transpose_block(xk, sb)
                for g in range(NG):
                    pk = ps_k.tile([P, SBLK], f32, tag="k")
                    for dc in range(DCH):
                        nc.tensor.matmul(
                            pk[:],
                            wk_r[:, dc, ts(g, P)],
                            xkT[:, dc, :],
                            start=(dc == 0),
                            stop=(dc == DCH - 1),
                        )
                    nc.vector.tensor_scalar_add(
                        khT[g][:, ts(sb, SBLK)], pk[:], bk_sb[:, g : g + 1]
                    )
                xvT = transpose_block(xv, sb)
                for ss in range(4):
                    pv = ps_k.tile([P, DLOC], f32, tag="k")
                    for dc in range(DCH):
                        nc.tensor.matmul(
                            pv[:],
                            xvT[:, dc, ts(ss, P)],
                            wv_r[:, dc, :],
                            start=(dc == 0),
                            stop=(dc == DCH - 1),
                        )
                    kt = sb * 4 + ss
                    nc.vector.tensor_copy(
                        vh4[:, kt, :, 0:64],
                        pv[:].rearrange("p (h e) -> p h e", e=64),
                    )

            # ---- phase B: attention + o_proj per q block ------------------
            for qb in range(NSB):
                xqT = transpose_block(xq, qb)
                qhT = qpool.tile([P, NG, SBLK], f32r)
                for g in range(NG):
                    pq = ps_k.tile([P, SBLK], f32, tag="k")
                    for dc in range(DCH):
                        nc.tensor.matmul(
                            pq[:],
                            wq_r[:, dc, ts(g, P)],
                            xqT[:, dc, :],
                            start=(dc == 0),
                            stop=(dc == DCH - 1),
                        )
                    nc.vector.tensor_scalar_add(
                        qhT[:, g, :], pq[:], bq_sb[:, g : g + 1]
                    )

                ctx2 = cpool.tile([P, NG, SBLK], f32r)
                for h in range(NH):
                    g, hr = h // 2, (h % 2) * 64
                    pc = ps_c.tile([P, SBLK], f32, tag="c")
                    for kt2 in range(NKT // 2):
                        # two key tiles per psum pair -> one big exp call
                        ps2 = ps_s.tile([P, 2 * SBLK], f32, tag="s")
                        for half in range(2):
                            kt = 2 * kt2 + half
                            nc.tensor.matmul(
                                ps2[:, ts(half, SBLK)],
                                khT[g][hr : hr + 64, ts(kt, P)],
                                qhT[hr : hr + 64, g, :],
                                start=True,
                                stop=True,
                            )
                        ex = epool.tile([P, 2 * SBLK], f32r)
                        nc.scalar.activation(
                            ex[:], ps2[:], AF.Exp, scale=0.125
                        )
                        for half in range(2):
                            kt = 2 * kt2 + half
                            nc.tensor.matmul(
                                pc[0:65, :],
                                vh_aug[:, kt, h * 65 : h * 65 + 65],
                                ex[:, ts(half, SBLK)],
                                start=(kt == 0),
                                stop=(kt == NKT - 1),
                            )
                    rc = rpool.tile([1, SBLK], f32)
                    nc.vector.reciprocal(rc[:], pc[64:65, :])
                    pb = ps_s.tile([64, SBLK], f32, tag="s", name="pb")
                    nc.tensor.matmul(
                        pb[:], ones_col[:], rc[:], start=True, stop=True
                    )
                    rb = rpool.tile([64, SBLK], f32, name="rb")
                    nc.vector.tensor_copy(rb[:], pb[:])
                    nc.vector.tensor_mul(
                        ctx2[hr : hr + 64, g, :], pc[0:64, :], rb[:]
                    )
                for g in range(NG):
                    nc.vector.tensor_scalar_add(
                        ctx2[:, g, :], ctx2[:, g, :], bv_sb[:, g : g + 1]
                    )

                for qs in range(4):
                    ost = opool.tile([P, D], f32)
                    for nch in range(2):
                        po = ps_t.tile([P, SBLK], f32, tag="t")
                        for g in range(NG):
                            nc.tensor.matmul(
                                po[:],
                                ctx2[:, g, ts(qs, P)],
                                wo_r[:, g, ts(nch, SBLK)],
                                start=(g == 0),
                                stop=(g == NG - 1),
                            )
                        nc.vector.tensor_copy(ost[:, ts(nch, SBLK)], po[:])
                    nc.sync.dma_start(
                        y[ds(qb * SBLK + qs * P, P), :], ost[:]
                    )

    import concourse.mybir as mybir

    _split_excess_waits(nc, mybir)
    return nc


def kernel(q, k, v, Wq, bq, Wk, bk, Wv, bv, Wo, bo):
    from concourse.bass_utils import run_bass_kernel_spmd

    q = np.asarray(q, dtype=np.float32)
    k = np.asarray(k, dtype=np.float32)
    v = np.asarray(v, dtype=np.float32)
    Wq = np.asarray(Wq, dtype=np.float32)
    Wk = np.asarray(Wk, dtype=np.float32)
    Wv = np.asarray(Wv, dtype=np.float32)
    Wo = np.asarray(Wo, dtype=np.float32)
    bq = np.asarray(bq, dtype=np.float32)
    bk = np.asarray(bk, dtype=np.float32)
    bv = np.asarray(bv, dtype=np.float32)
    bo = np.asarray(bo, dtype=np.float32)

    if "nc" not in _program_cache:
        _program_cache["nc"] = _build_program()
    nc = _program_cache["nc"]

    in_maps = []
    for c in range(8):
        b, hg = c // 4, c % 4
        cols = slice(DLOC * hg, DLOC * (hg + 1))
        in_maps.append(
            {
                "xq": np.ascontiguousarray(q[b]),
                "xk": np.ascontiguousarray(k[b]),
                "xv": np.ascontiguousarray(v[b]),
                "wq": np.ascontiguousarray(Wq[:, cols]),
                "wk": np.ascontiguousarray(Wk[:, cols]),
                "wv": np.ascontiguousarray(Wv[:, cols]),
                "wo": np.ascontiguousarray(Wo[cols, :]),
                "bq": np.ascontiguousarray(bq[cols]),
                "bk": np.ascontiguousarray(bk[cols]),
                "bv": np.ascontiguousarray(bv[cols]),
            }
        )

    global _last_in_maps
    _last_in_maps = in_maps

    res = run_bass_kernel_spmd(nc, in_maps, list(range(8)))

    out = np.empty((B, S, D), np.float32)
    for b in range(B):
        acc = res.results[4 * b]["y"].astype(np.float32).copy()
        for hg in range(1, 4):
            acc += res.results[4 * b + hg]["y"]
        out[b] = acc + bo[None, :]
    return out
